# revision 12
# baseline (speedup 1.0000x reference)
"""BiMamba adapter Trainium2 kernel (instruction-count-optimized rewrite).

Sharding: 8 cores = (batch 2) x (direction 2) x (d_inner half 2), SPMD.
This deployment is instruction-dispatch-bound (~47-73us per matmul,
~65us per DVE op, ~75us per Act op, ~11us per DMA, nearly independent
of tile size), so the design minimizes instruction count per engine:

- in_proj / xproj / out_proj run in fp8e4 DoubleRow mode (2 contraction
  k-tiles per PE instruction).  Precision is safe because the final
  output is residual-dominated (out = x + small adapter delta).
- LN runs on host-pre-transposed x via Pool partition_all_reduce plus
  stride-0-broadcast DVE fixups: no PE transposes, no per-token chains.
- The causal conv runs batched across channel chunks with stride-0
  weight-pattern views.
- PSUM is evacuated in (128, 2048) four-bank tiles (1 Act each).
- Scan B/C broadcast: 1 staging DMA + 1 Pool partition_broadcast per
  state instead of 13 doubling DMAs per row.
- dt chunk starts are poisoned (dt=60000 -> dA=0) so one wide scan
  instruction handles 6 chunks at once.
"""
import numpy as np
import ml_dtypes

import concourse.bass as bass
import concourse.bacc as bacc
import concourse.tile as tile
from concourse import mybir
from concourse import bass_isa
from concourse.bass_utils import run_bass_kernel_spmd

F8 = mybir.dt.float8e4
F16 = mybir.dt.float16
F32 = mybir.dt.float32
OP = mybir.AluOpType
AF = mybir.ActivationFunctionType
DR = mybir.MatmulPerfMode.DoubleRow
NPF8 = ml_dtypes.float8_e4m3

L = 2048
DM = 768
DI = 1536
DH = 768
DTR = 48
NS = 16
NDM = 6            # d_model chunks
NDU = 12           # full d_inner chunks
NDH = 6            # half d_inner chunks
KC = 4
FC = 512
NFC = L // FC      # 4
WB = NDH * L       # 12288
NUC = 6            # u chunks computed per core (own half only)
CP = L + 4         # padded conv chunk pitch
SW = 64.0          # fp8 scale: in_proj weights
SX = 64.0          # fp8 scale: xproj weights
SU = 32.0          # fp8 scale: u
SY = 64.0          # fp8 scale: gated y
SW2 = 256.0        # fp8 scale: out_proj weights
SBC = 16.0         # scan range scale on v, B, C (keeps f16 normal)


def _build_program(rep=1, variant="full"):
    nc = bacc.Bacc("TRN2", target_bir_lowering=False, debug=False, num_devices=8)

    def din(name, shape, dt):
        return nc.dram_tensor(name, shape, dt, kind="ExternalInput").ap()

    aps = dict(
        xt=din("xt", [128, NDM * L], F16),
        wu8=din("wu8", [128, NDM * (DH + DH)], F8),
        buz=din("buz", [128, NUC + NDH], F32),
        convw=din("convw", [128, NUC * KC], F16),
        convb=din("convb", [128, NUC], F16),
        xp8=din("xp8", [128, NUC * 80], F8),
        dtw=din("dtw", [DTR, DH], F16),
        dtb=din("dtb", [128, NDH], F32),
        dvec=din("dvec", [128, NDH], F16),
        w28=din("w28", [128, NDH * DM], F8),
        qout=nc.dram_tensor("q", [DM, L], F32, kind="ExternalOutput").ap(),
    )
    aps["sgd"] = nc.dram_tensor("sgd", [128, WB], F16).ap()

    with tile.TileContext(nc) as tc:
        for _ in range(rep):
            _body(tc, nc, aps, variant)
    nc.compile()
    return nc


def _body(tc, nc, aps, variant="full"):
    qout = aps["qout"]

    with tc.tile_pool(name="pers", bufs=1) as pers:
        # live through the scan
        dt_big = pers.tile([128, WB], F16, tag="dt_big", name="dt_big")
        v_big = pers.tile([128, WB], F16, tag="v_big", name="v_big")
        yacc = pers.tile([128, WB], F16, tag="yacc", name="yacc")
        xd16 = pers.tile([80, L], F16, tag="xd16", name="xd16")
        bc8 = pers.tile([32, L], F8, tag="bc8", name="bc8")
        eps_sb = pers.tile([128, 1], F32, tag="eps", name="eps_sb")
        nc.vector.memset(eps_sb[:], 1e-5)

        # ================= phase A =================
        with tc.tile_pool(name="pA", bufs=1) as pA:
            buz = pA.tile([128, NUC + NDH], F32, tag="buz", name="buz")
            nc.sync.dma_start(buz[:], aps["buz"])
            convw = pA.tile([128, NUC * KC], F16, tag="convw", name="convw")
            nc.sync.dma_start(convw[:], aps["convw"])
            convb = pA.tile([128, NUC], F16, tag="convb", name="convb")
            nc.sync.dma_start(convb[:], aps["convb"])
            xp8 = pA.tile([128, NUC * 80], F8, tag="xp8", name="xp8")
            nc.sync.dma_start(xp8[:], aps["xp8"])
            dtw = pA.tile([DTR, DH], F16, tag="dtw", name="dtw")
            nc.sync.dma_start(dtw[:], aps["dtw"])
            dtb = pA.tile([128, NDH], F32, tag="dtb", name="dtb")
            nc.sync.dma_start(dtb[:], aps["dtb"])
            dvec = pA.tile([128, NDH], F16, tag="dvec", name="dvec")
            nc.sync.dma_start(dvec[:], aps["dvec"])
            # xn8 shares f8buf with u8 (same 12288-elem footprint)
            f8buf = pA.tile([128, NDM * L], F8, tag="f8buf", name="f8buf")
            xn8 = f8buf[:, :]

            # ---- layernorm (tokens along free axis) ----
            with tc.tile_pool(name="lns", bufs=1) as lns:
                xt = lns.tile([128, NDM * L], F16, tag="xt", name="xt")
                nc.sync.dma_start(xt[:], aps["xt"])
                s1 = lns.tile([128, NDM * L], F16, tag="s1", name="s1")
                s2 = lns.tile([128, NDM * L], F16, tag="s2", name="s2")
                m1 = lns.tile([128, L], F32, tag="m1", name="m1")
                m2 = lns.tile([128, L], F32, tag="m2", name="m2")
                mu2 = lns.tile([128, L], F32, tag="mu2", name="mu2")

                nc.gpsimd.partition_all_reduce(s2[:], xt[:], 128,
                                               bass_isa.ReduceOp.add)
                nc.vector.tensor_reduce(
                    m1[:], s2[:].rearrange("p (c t) -> p t c", c=NDM),
                    mybir.AxisListType.X, OP.add)
                nc.scalar.activation(s1[:], xt[:], AF.Square)
                nc.gpsimd.partition_all_reduce(s2[:], s1[:], 128,
                                               bass_isa.ReduceOp.add)
                nc.vector.tensor_reduce(
                    m2[:], s2[:].rearrange("p (c t) -> p t c", c=NDM),
                    mybir.AxisListType.X, OP.add)
                nc.vector.tensor_scalar_mul(m1[:], m1[:], 1.0 / DM)
                nc.vector.tensor_tensor(mu2[:], m1[:], m1[:], OP.mult)
                # var -> mu2 ; sdev -> m2 ; rstd -> mu2
                nc.vector.scalar_tensor_tensor(mu2[:], m2[:], 1.0 / DM,
                                               mu2[:], OP.mult, OP.subtract)
                nc.scalar.activation(m2[:], mu2[:], AF.Sqrt, bias=eps_sb[:])
                nc.vector.reciprocal(mu2[:], m2[:])
                muv = m1[:].unsqueeze(1).broadcast_to((128, NDM, L))
                xtv = xt[:].rearrange("p (c t) -> p c t", c=NDM)
                nc.vector.scalar_tensor_tensor(xtv, muv, -1.0, xtv,
                                               OP.mult, OP.add)
                rsv = mu2[:].unsqueeze(1).broadcast_to((128, NDM, L))
                nc.vector.tensor_tensor(
                    xn8.rearrange("p (c t) -> p c t", c=NDM), xtv, rsv,
                    OP.mult)

            # ---- in_proj (fp8 DoubleRow) + conv + xproj + dt ----
            with tc.tile_pool(name="mid", bufs=1) as mid:
                u_pre = mid.tile([128, NUC * CP], F8, tag="u_pre",
                                 name="u_pre")
                nc.vector.memset(u_pre[:], 0.0)
                xnv = xn8.rearrange("p (c t) -> p c t", c=NDM)

                with tc.tile_pool(name="pipa", bufs=1) as pipa, \
                     tc.tile_pool(name="psA", bufs=2,
                                  space=bass.MemorySpace.PSUM) as psA:
                    wu8 = pipa.tile([128, NDM * (DH + DH)], F8, tag="wu8",
                                    name="wu8")
                    nc.sync.dma_start(wu8[:], aps["wu8"])
                    sg = pipa.tile([128, WB], F16, tag="sg", name="sg")
                    wuv = wu8[:].rearrange("p (c m) -> p c m", c=NDM)
                    for ic in range(NUC + NDH):
                        pb = psA.tile([128, L], F32, tag="pbig", name="pb")
                        if "nomm" not in variant:
                            for fc in range(NFC):
                                for j in range(3):
                                    nc.tensor.matmul(
                                        pb[:, fc * FC:(fc + 1) * FC],
                                        wuv[:, 2 * j:2 * j + 2,
                                            ic * 128:(ic + 1) * 128],
                                        xnv[:, 2 * j:2 * j + 2,
                                            fc * FC:(fc + 1) * FC],
                                        start=(j == 0), stop=(j == 2),
                                        perf_mode=DR)
                        if ic < NUC:
                            nc.scalar.activation(
                                u_pre[:, ic * CP + 4:ic * CP + 4 + L],
                                pb[:], AF.Identity, scale=1.0 / SW,
                                bias=buz[:, ic:ic + 1])
                        else:
                            zc = ic - NUC
                            nc.scalar.activation(
                                sg[:, zc * L:(zc + 1) * L], pb[:], AF.Silu,
                                scale=1.0 / SW,
                                bias=buz[:, NUC + zc:NUC + zc + 1])
                    nc.sync.dma_start(aps["sgd"], sg[:])

                # ---- causal conv, batched in two 6-chunk halves ----
                with tc.tile_pool(name="pconv", bufs=1) as pcv:
                    u_big = pcv.tile([128, NUC * L], F16, tag="u_big",
                                     name="u_big")
                    tmp = pcv.tile([128, NUC * L], F16, tag="tmp",
                                   name="tmp")
                    cwv = convw[:].rearrange("p (c k) -> p c k", c=NUC)
                    upva = u_pre[:].rearrange("p (c t) -> p c t", c=NUC)
                    ubva = u_big[:].rearrange("p (c t) -> p c t", c=NUC)
                    tmv = tmp[:].rearrange("p (c t) -> p c t", c=NUC)
                    for k in range(KC):
                        wp = cwv[:, :, k:k + 1].broadcast_to((128, NUC, L))
                        usrc = upva[:, :, 1 + k:1 + k + L]
                        if k == 0:
                            nc.vector.tensor_tensor(ubva, usrc, wp, OP.mult)
                        else:
                            nc.vector.tensor_tensor(tmv, usrc, wp, OP.mult)
                            nc.vector.tensor_tensor(ubva, ubva, tmv, OP.add)
                    cbp = convb[:].unsqueeze(2).broadcast_to((128, NUC, L))
                    nc.vector.tensor_tensor(ubva, ubva, cbp, OP.add)
                    nc.scalar.activation(u_big[:], u_big[:], AF.Silu)

                    # u8 overwrites f8buf (WAW after xn8's last read)
                    nc.vector.tensor_scalar_mul(f8buf[:], u_big[:], SU)

                    # yacc init from own half
                    dvp = dvec[:].unsqueeze(2).broadcast_to((128, NDH, L))
                    nc.vector.tensor_tensor(
                        yacc[:].rearrange("p (c t) -> p c t", c=NDH),
                        ubva, dvp, OP.mult)

                    # ---- xproj (fp8 DoubleRow, own half) + pair AllReduce ----
                    u8v = f8buf[:].rearrange("p (c t) -> p c t", c=NUC)
                    xpv = xp8[:].rearrange("p (c m) -> p c m", c=NUC)
                    with tc.tile_pool(name="psX", bufs=1,
                                      space=bass.MemorySpace.PSUM) as psX:
                        px = psX.tile([80, L], F32, tag="px", name="px")
                        if "nomm" not in variant:
                            for fc in range(NFC):
                                for j in range(NUC // 2):
                                    nc.tensor.matmul(
                                        px[:, fc * FC:(fc + 1) * FC],
                                        xpv[:, 2 * j:2 * j + 2, :],
                                        u8v[:, 2 * j:2 * j + 2,
                                            fc * FC:(fc + 1) * FC],
                                        start=(j == 0),
                                        stop=(j == NUC // 2 - 1),
                                        perf_mode=DR)
                        xdP = pcv.tile([80, L], F16, tag="xdP", name="xdP")
                        nc.scalar.activation(xdP[:], px[:], AF.Copy,
                                             scale=1.0 / (SX * SU))
                    with tc.tile_pool(name="dramb", bufs=1,
                                      space="DRAM") as dramb:
                        ib = dramb.tile([80, L], F16, name="ib")
                        ob = dramb.tile([80, L], F16, name="ob")
                        nc.gpsimd.dma_start(ib[:], xdP[:])
                        nc.gpsimd.collective_compute(
                            "AllReduce", OP.add,
                            replica_groups=[[0, 1], [2, 3], [4, 5], [6, 7]],
                            ins=[ib.opt()], outs=[ob.opt()])
                        nc.gpsimd.dma_start(xd16[:], ob[:])
                    bcA = pcv.tile([80, L], F8, tag="bcA", name="bcA")
                    nc.scalar.activation(bcA[:], xd16[:], AF.Copy,
                                         scale=SBC)
                    nc.sync.dma_start(bc8[:], bcA[DTR:DTR + 32, :])

                    # ---- dt = softplus(dtlow @ dtw + dtb) ----
                    with tc.tile_pool(name="psD", bufs=2,
                                      space=bass.MemorySpace.PSUM) as psD:
                        for mc in range(NDH):
                            pd = psD.tile([128, L], F32, tag="pd", name="pd")
                            if "nomm" not in variant:
                                for fc in range(NFC):
                                    nc.tensor.matmul(
                                        pd[:, fc * FC:(fc + 1) * FC],
                                        dtw[:, mc * 128:(mc + 1) * 128],
                                        xd16[0:DTR, fc * FC:(fc + 1) * FC],
                                        start=True, stop=True)
                            nc.scalar.activation(
                                dt_big[:, mc * L:(mc + 1) * L], pd[:],
                                AF.Exp, bias=dtb[:, mc:mc + 1])
                    nc.scalar.activation(dt_big[:], dt_big[:], AF.Ln,
                                         bias=1.0)
                    nc.vector.scalar_tensor_tensor(
                        v_big[:], dt_big[:], SBC, u_big[:],
                        OP.mult, OP.mult)

        # poison chunk starts: dA -> 0 resets the batched scan exactly
        pois = dt_big[:].rearrange("p (c t) -> p c t", t=L)[:, :, 0:1]
        nc.vector.memset(pois, 60000.0)

        # ================= phase B: scan =================
        with tc.tile_pool(name="sw", bufs=1) as swp, \
             tc.tile_pool(name="sw2", bufs=2) as swp2:
            for n in range(0 if "noscan" in variant else NS):
                stg = swp.tile([1, 2 * L], F8, tag="stg", name="stg")
                nc.sync.dma_start(
                    stg[:].rearrange("p (a t) -> p a t", a=2),
                    bc8[n:n + 17:16, :])
                bcb = swp.tile([128, 2 * WB], F8, tag="bcb", name="bcb")
                inv = stg[:].rearrange("p (a t) -> p a t", a=2).unsqueeze(
                    2).broadcast_to((1, 2, NDH, L))
                nc.gpsimd.partition_broadcast(bcb[:], inv, channels=128)
                at = swp2.tile([128, WB], F16, tag="at", name="at")
                nc.scalar.activation(at[:], dt_big[:], AF.Exp,
                                     scale=-float(n + 1))
                bb = swp.tile([128, WB], F16, tag="bb", name="bb")
                nc.vector.tensor_tensor(bb[:], v_big[:], bcb[:, 0:WB],
                                        OP.mult)
                ht = swp.tile([128, WB], F16, tag="ht", name="ht")
                if variant == "nosc":
                    nc.vector.tensor_tensor(ht[:], at[:], bb[:], OP.mult)
                else:
                    nc.vector.tensor_tensor_scan(ht[:], at[:], bb[:], 0.0,
                                                 OP.mult, OP.add)
                nc.vector.tensor_tensor(bb[:], ht[:], bcb[:, WB:], OP.mult)
                nc.gpsimd.tensor_tensor(yacc[:], yacc[:], bb[:], OP.add)

        # ================= phase C: gate + out_proj =================
        with tc.tile_pool(name="pc", bufs=1) as pc, \
             tc.tile_pool(name="psC", bufs=2,
                          space=bass.MemorySpace.PSUM) as psC:
            sgr = pc.tile([128, WB], F16, tag="sgr", name="sgr")
            nc.sync.dma_start(sgr[:], aps["sgd"])
            w28 = pc.tile([128, NDH * DM], F8, tag="w28", name="w28")
            nc.sync.dma_start(w28[:], aps["w28"])
            y8 = pc.tile([128, WB], F8, tag="y8", name="y8")
            nc.vector.scalar_tensor_tensor(y8[:], yacc[:],
                                           SY / (SBC * SBC * SBC),
                                           sgr[:], OP.mult, OP.mult)
            y8v = y8[:].rearrange("p (c t) -> p c t", c=NDH)
            w2v = w28[:].rearrange("p (c m) -> p c m", c=NDH)
            for mc in range(NDM):
                pq = psC.tile([128, L], F32, tag="pq", name="pq")
                if "nomm" not in variant:
                    for fc in range(NFC):
                        for j in range(3):
                            nc.tensor.matmul(
                                pq[:, fc * FC:(fc + 1) * FC],
                                w2v[:, 2 * j:2 * j + 2,
                                    mc * 128:(mc + 1) * 128],
                                y8v[:, 2 * j:2 * j + 2,
                                    fc * FC:(fc + 1) * FC],
                                start=(j == 0), stop=(j == 2), perf_mode=DR)
                qsb = pc.tile([128, L], F32, tag="qsb", name="qsb", bufs=2)
                nc.scalar.activation(qsb[:], pq[:], AF.Copy,
                                     scale=1.0 / (SY * SW2))
                nc.sync.dma_start(qout[mc * 128:(mc + 1) * 128, :], qsb[:])


_CACHE = {}


def _get_program(rep=1, variant="full"):
    key = (rep, variant)
    if key not in _CACHE:
        _CACHE[key] = _build_program(rep, variant)
    return _CACHE[key]


def _chunked(a, nch, pitch=128):
    """(nch*pitch, F) -> (pitch, nch*F) chunk-major layout."""
    f = a.shape[1]
    return np.ascontiguousarray(
        a.reshape(nch, pitch, f).transpose(1, 0, 2).reshape(pitch, nch * f))


def _prep_core_inputs(inp, b, d, half):
    f32, f16 = np.float32, np.float16
    pref = "mf" if d == 0 else "mb"
    g = lambda k: np.asarray(inp[f"{pref}_{k}"], f32)
    ln_w = np.asarray(inp["ln_w"], f32)
    ln_b = np.asarray(inp["ln_b"], f32)
    in_w = g("in_w")
    x = np.asarray(inp["x"], f32)[b]
    if d == 1:
        x = x[::-1]
    perm = np.concatenate([np.arange(half * DH, (half + 1) * DH),
                           np.arange((1 - half) * DH, (2 - half) * DH)])
    hs = slice(half * DH, (half + 1) * DH)
    wu = in_w[0:DI][perm]                        # (DI, DM)
    wz = in_w[DI + half * DH:DI + (half + 1) * DH]
    A = -np.exp(g("A_log")[hs])
    assert np.abs(A + np.arange(1, NS + 1)).max() < 1e-4, \
        "kernel assumes A[:, n] == -(n+1)"

    wu_own = wu[0:DH]                            # own half rows first (perm)
    wuz = np.concatenate([wu_own, wz], 0)        # (1536, DM)
    wuzT = (wuz * ln_w[None, :]).T               # (DM, 1536)
    bu = wu_own @ ln_b
    bz = wz @ ln_b
    w2 = (np.asarray(inp["proj_w"], f32)[:, d * DM:(d + 1) * DM]
          @ g("out_w")[:, hs])                   # (DM, DH)

    return {
        "xt": _chunked(np.ascontiguousarray(x.T), NDM).astype(f16),
        "wu8": _chunked(wuzT * SW, NDM).astype(NPF8),
        "buz": np.ascontiguousarray(np.concatenate(
            [bu.reshape(NUC, 128), bz.reshape(NDH, 128)]).T, f32),
        "convw": _chunked(g("conv_w")[hs], NUC).astype(f16),
        "convb": np.ascontiguousarray(
            g("conv_b")[hs].reshape(NUC, 128).T).astype(f16),
        "xp8": _chunked(np.ascontiguousarray(g("xproj_w").T[hs]) * SX,
                        NUC).astype(NPF8),
        "dtw": np.ascontiguousarray(g("dt_w")[hs].T).astype(f16),
        "dtb": np.ascontiguousarray(g("dt_b")[hs].reshape(NDH, 128).T, f32),
        "dvec": np.ascontiguousarray(
            g("D")[hs].reshape(NDH, 128).T * SBC ** 3).astype(f16),
        "w28": _chunked(np.ascontiguousarray(w2.T) * SW2, NDH).astype(NPF8),
    }


def _run(inp, rep=1, trace=False, variant="full"):
    nc = _get_program(rep, variant)
    in_maps = []
    for c in range(8):
        b, d, half = c >> 2, (c >> 1) & 1, c & 1
        in_maps.append(_prep_core_inputs(inp, b, d, half))
    for attempt in range(3):
        try:
            return run_bass_kernel_spmd(nc, in_maps, list(range(8)),
                                        trace=trace)
        except Exception:
            if attempt == 2:
                raise


def kernel(**inputs):
    res = _run(inputs, rep=1)
    x = np.asarray(inputs["x"], np.float32)
    proj_b = np.asarray(inputs["proj_b"], np.float32)
    out = np.empty((2, L, DM), np.float32)
    for b in range(2):
        acc = x[b] + proj_b
        for d in range(2):
            for half in range(2):
                c = (b << 2) | (d << 1) | half
                q = res.results[c]["q"].T          # (L, DM)
                if d == 1:
                    q = q[::-1]
                acc = acc + q
        out[b] = acc
    return out


if __name__ == "__main__":
    nc = _get_program(1)
    print("build ok")
